# revision 34
# baseline (speedup 1.0000x reference)
"""AutoCorrelation (Autoformer) Bass kernel for Trainium2, 8 NeuronCores.

Inputs (full): queries/keys/values [4, 207, 96, 8, 64] f32, attn_mask scalar.
Outputs: tuple (V, corr), each [4, 207, 96, 8, 64] f32.

Strategy: flatten (B,N) -> 828 slices of [L=96, H*E=512], sharded over 8
cores (pad to 832 = 8*104). Per slice:
  - corr path needs ~fp32-exact values (top-k ranks swap vs the fp32
    reference otherwise). q,k are split HOST-side into fp16 hi/lo pairs;
    the forward DFT runs as 3 accumulating fp16 matmuls (exact fp16
    products, fp32 PSUM accumulation) -> fp32-quality at 1 cyc/row
    instead of fp32's 4. The complex product runs on the Pool engine in
    fp32; the inverse DFT emits corr TRANSPOSED ([chan, lag]) via 8 fp32
    chunk matmuls so top-k can reduce along the free axis; the corr
    output is DMA'd out transposed (fp16) and untransposed on the host.
  - top-k=4 via DVE max8/max_index straight from PSUM; softmax on
    DVE/ACT; weighted one-hot built per-tap with fused tensor_scalar
    (is_equal, mult) in fp16; the 4 taps are tap-summed for free by
    ACCUMULATING the 16 one-hot transposes into PSUM (start/stop).
  - V path fully fp16: UF/VF DFTs, complex product via swapped-half AP
    reads (no swap copy), inverse DFT -> vo fp16.
"""

import math
import sys

sys.path.insert(0, "/opt/trn_rl_repo")

import numpy as np

import concourse.bacc as bacc
import concourse.bass as bass
import concourse.mybir as mybir
from concourse import tile
from concourse.bass_utils import run_bass_kernel_spmd

B, N, L, H, E = 4, 207, 96, 8, 64
HE = H * E          # 512
F = L // 2 + 1      # 49
PW = 128            # padded freq partitions: Re 0..48, Im 64..112
IMG = 64
TOPK = 4            # int(log(96))
NCORES = 8
NSLICES = B * N                      # 828
S = math.ceil(NSLICES / NCORES)      # 104
NT = HE // 128                       # 4 channel tiles of 128
EST_DVE = 7                          # of 16 one-hot builds, how many on DVE

FP = mybir.dt.float32
F16 = mybir.dt.float16
U32 = mybir.dt.uint32
AF = mybir.ActivationFunctionType
ALU = mybir.AluOpType
F16_NP = np.float16


def _consts():
    t = np.arange(L)[:, None]
    f = np.arange(F)[None, :]
    C = np.cos(2 * np.pi * f * t / L)
    Sm = np.sin(2 * np.pi * f * t / L)
    Z15 = np.zeros((L, 64 - F))
    Dq = np.concatenate([C, Z15, -Sm, Z15], axis=1).astype(np.float32)  # [96,128]
    Dh = Dq.astype(F16_NP)
    Dl = (Dq - Dh.astype(np.float32)).astype(F16_NP)
    wf = np.full(F, 2.0)
    wf[0] = 1.0
    wf[F - 1] = 1.0
    tau = np.arange(L)[None, :]
    fc = np.arange(F)[:, None]
    IC = (wf[:, None] / L) * np.cos(2 * np.pi * fc * tau / L)   # [49, 96]
    ISn = (wf[:, None] / L) * np.sin(2 * np.pi * fc * tau / L)
    Z15r = np.zeros((64 - F, L))
    M1 = np.concatenate([IC, Z15r, IC, Z15r], axis=0).astype(np.float32)   # [128,96]
    M2P = np.concatenate([ISn, Z15r, -ISn, Z15r], axis=0).astype(np.float32)
    iota = np.tile(np.arange(L, dtype=np.float32), (128, 1))    # [128, 96]
    ident = np.eye(128, dtype=np.float32)
    # partition-swapped DFT: VFsw = Dq16sw.T @ v lands as [Vi; Vr]
    Dqsw = np.concatenate([Dq[:, IMG:], Dq[:, :IMG]], axis=1)
    # g2 = UF * VFsw = [Ur*Vi; Ui*Vr] -> inverse weights [-ISn; +ISn]
    M2gn = np.concatenate([-ISn, Z15r, ISn, Z15r], axis=0).astype(np.float32)
    f32 = dict(M1=M1, M2P=M2P)
    b16 = dict(Dh=Dh, Dl=Dl, Dq16=Dq.astype(F16_NP),
               Dq16sw=Dqsw.astype(F16_NP),
               M1g=M1.astype(F16_NP), M2g=M2gn.astype(F16_NP),
               iota16=iota.astype(F16_NP), ident16=ident.astype(F16_NP))
    return f32, b16


def _build_program(n_slices):
    nc = bacc.Bacc("TRN2", target_bir_lowering=False, debug=False,
                   num_devices=NCORES)
    qk = nc.dram_tensor("qk", [n_slices, L, 4 * HE], F16, kind="ExternalInput")
    vs = nc.dram_tensor("vs", [n_slices, L, HE], F16, kind="ExternalInput")
    cf32, cb16 = _consts()
    cdram = {}
    for kk, vv in cf32.items():
        cdram[kk] = nc.dram_tensor(kk, list(vv.shape), FP, kind="ExternalInput")
    for kk, vv in cb16.items():
        cdram[kk] = nc.dram_tensor(kk, list(vv.shape), F16, kind="ExternalInput")
    co = nc.dram_tensor("co", [n_slices, PW, NT * L], F16, kind="ExternalOutput")
    vo = nc.dram_tensor("vo", [n_slices, L, HE], F16, kind="ExternalOutput")

    with tile.TileContext(nc) as tc:
        with (
            tc.tile_pool(name="const", bufs=1) as cpool,
            tc.tile_pool(name="ioqk", bufs=6) as ioqk,
            tc.tile_pool(name="iov", bufs=12) as iov,
            tc.tile_pool(name="work", bufs=4) as wk,
            tc.tile_pool(name="small", bufs=6) as sm,
            tc.tile_pool(name="out", bufs=6) as op,
            tc.tile_pool(name="psQK", bufs=1, space="PSUM") as psQK,
            tc.tile_pool(name="psCT", bufs=1, space="PSUM") as psCT,
            tc.tile_pool(name="psUT", bufs=1, space="PSUM") as psUT,
            tc.tile_pool(name="psUV", bufs=1, space="PSUM") as psUV,
            tc.tile_pool(name="psAG", bufs=1, space="PSUM") as psAG,
        ):
            cb = {}
            for kk, vv in cf32.items():
                cb[kk] = cpool.tile(list(vv.shape), FP, tag=kk, name=kk)
                nc.sync.dma_start(out=cb[kk][:], in_=cdram[kk].ap())
            for kk, vv in cb16.items():
                cb[kk] = cpool.tile(list(vv.shape), F16, tag=kk, name=kk)
                nc.sync.dma_start(out=cb[kk][:], in_=cdram[kk].ap())

            tl = [dict() for _ in range(n_slices)]

            def st0(s, d):
                # DMA in
                d["qkt"] = ioqk.tile([L, 4 * HE], F16, tag="qkt", name="qkt")
                d["vt"] = iov.tile([L, HE], F16, tag="vt", name="vt")
                nc.sync.dma_start(out=d["qkt"][:], in_=qk.ap()[s])
                nc.sync.dma_start(out=d["vt"][:], in_=vs.ap()[s])

            def st1(s, d):
                # forward DFT of q,k: 3 fp16 matmuls per half, fp32 accum.
                # qkt cols: [qh | ql | kh | kl]; PSUM out is bank-limited
                # to 512 f32, so QF and KF get separate chains.
                qkt = d["qkt"]
                QKF = d["QKF"] = psQK.tile([PW, 2 * HE], FP, tag="QKF",
                                           name="QKF")
                for half in range(2):
                    hi = qkt[:, 2 * half * HE:(2 * half + 1) * HE]
                    lo = qkt[:, (2 * half + 1) * HE:(2 * half + 2) * HE]
                    o = QKF[:, half * HE:(half + 1) * HE]
                    nc.tensor.matmul(o, cb["Dh"][:], hi, start=True, stop=False)
                    nc.tensor.matmul(o, cb["Dh"][:], lo, start=False, stop=False)
                    nc.tensor.matmul(o, cb["Dl"][:], hi, start=False, stop=True)

            def st2(s, d):
                # complex products: m1 on Pool (via SBUF stage), m2 on DVE
                QKF = d["QKF"]
                qkf = wk.tile([PW, 2 * HE], FP, tag="qkf", name="qkf")
                nc.scalar.copy(qkf[:], QKF[:])
                m12 = d["m12"] = wk.tile([PW, 2 * HE], FP, tag="m12",
                                         name="m12")
                nc.gpsimd.tensor_mul(m12[:, 0:HE], qkf[:, 0:HE],
                                     qkf[:, HE:2 * HE])
                # partition-crossed ops stay on DVE, mixing SBUF x PSUM
                # operands (the equal-base rule applies to SB x SB pairs)
                nc.vector.tensor_mul(m12[:IMG, HE:2 * HE],
                                     qkf[:IMG, 0:HE], QKF[IMG:, HE:2 * HE])
                nc.vector.tensor_mul(m12[IMG:, HE:2 * HE],
                                     qkf[IMG:, 0:HE], QKF[:IMG, HE:2 * HE])

            def st3(s, d):
                # inverse DFT directly transposed: corrT [128c, 4*96]
                m12 = d["m12"]
                corrT = d["corrT"] = psCT.tile([PW, NT * L], FP, tag="corrT",
                                               name="corrT")
                for T in range(NT):
                    nc.tensor.matmul(corrT[:, T * L:(T + 1) * L],
                                     m12[:, T * 128:(T + 1) * 128],
                                     cb["M1"][:], start=True, stop=False)
                    nc.tensor.matmul(corrT[:, T * L:(T + 1) * L],
                                     m12[:, HE + T * 128:HE + (T + 1) * 128],
                                     cb["M2P"][:], start=False, stop=True)

            def st4(s, d):
                # stage corrT out of PSUM immediately (frees the bufs=1 PSUM
                # for the next slice -- this was the binding cycle), then
                # top-k from SBUF and the fp16 corr output copy on Pool
                corrT = d["corrT"]
                ctsb = d["ctsb"] = wk.tile([PW, NT * L], FP, tag="ctsb",
                                           name="ctsb")
                nc.scalar.copy(ctsb[:], corrT[:])
                co16 = op.tile([PW, NT * L], F16, tag="co16", name="co16")
                nc.gpsimd.tensor_copy(co16[:], ctsb[:])
                nc.sync.dma_start(out=co.ap()[s], in_=co16[:])
                t8v = d["t8v"] = sm.tile([128, 8 * NT], FP, tag="t8v",
                                         name="t8v")
                t8i = d["t8i"] = sm.tile([128, 8 * NT], U32, tag="t8i",
                                         name="t8i")
                for T in range(NT):
                    nc.vector.max(t8v[:, T * 8:(T + 1) * 8],
                                  ctsb[:, T * L:(T + 1) * L])
                    nc.vector.max_index(t8i[:, T * 8:(T + 1) * 8],
                                        t8v[:, T * 8:(T + 1) * 8],
                                        ctsb[:, T * L:(T + 1) * L])

            def st5(s, d):
                # batched softmax over 4 taps x 4 tiles
                t8v, t8i = d["t8v"], d["t8i"]
                wexp = sm.tile([128, TOPK * NT], FP, tag="wexp", name="wexp")
                wsum = sm.tile([128, NT], FP, tag="wsum", name="wsum")
                wrec = sm.tile([128, NT], FP, tag="wrec", name="wrec")
                wnrm = d["wnrm"] = sm.tile([128, TOPK * NT], FP, tag="wnrm",
                                           name="wnrm")
                dF = d["dF"] = sm.tile([128, TOPK * NT], FP, tag="dF",
                                       name="dF")
                t8v_4 = t8v[:].rearrange("p (t e) -> p t e", e=8)[:, :, 0:TOPK]
                nc.scalar.activation(
                    wexp[:].rearrange("p (t e) -> p t e", e=TOPK),
                    t8v_4, AF.Exp)
                nc.vector.tensor_reduce(
                    wsum[:], wexp[:].rearrange("p (t e) -> p t e", e=TOPK),
                    axis=mybir.AxisListType.X, op=ALU.add)
                nc.vector.reciprocal(wrec[:], wsum[:])
                wrec_ap = wrec[:]
                wrec_b = bass.AP(wrec_ap.tensor, wrec_ap.offset,
                                 [wrec_ap.ap[0], [1, NT], [0, TOPK]])
                nc.gpsimd.tensor_mul(
                    wnrm[:].rearrange("p (t e) -> p t e", e=TOPK),
                    wexp[:].rearrange("p (t e) -> p t e", e=TOPK), wrec_b)
                t8i_4 = t8i[:].rearrange("p (t e) -> p t e", e=8)[:, :, 0:TOPK]
                nc.gpsimd.tensor_copy(
                    dF[:].rearrange("p (t e) -> p t e", e=TOPK), t8i_4)

            def st6(s, d):
                # weighted one-hot est[c, (T,i,lag)] fp16, DVE/Pool split
                wnrm, dF = d["wnrm"], d["dF"]
                est = d["est"] = wk.tile([128, NT * TOPK * L], F16, tag="est",
                                         name="est")
                dve_js = {int(i * 16 / max(EST_DVE, 1)) for i in range(EST_DVE)}
                for T in range(NT):
                    for i in range(TOPK):
                        j = T * TOPK + i
                        eng = nc.vector if j in dve_js else nc.gpsimd
                        eng.tensor_scalar(
                            out=est[:, j * L:(j + 1) * L],
                            in0=cb["iota16"][:],
                            scalar1=dF[:, j:j + 1],
                            scalar2=wnrm[:, j:j + 1],
                            op0=ALU.is_equal,
                            op1=ALU.mult)

            def st7(s, d):
                # tap-merge for free: est_chunk.T @ I accumulated in PSUM
                est = d["est"]
                uT = d["uT"] = psUT.tile([L, HE], FP, tag="uT", name="uT")
                for T in range(NT):
                    for i in range(TOPK):
                        j = T * TOPK + i
                        nc.tensor.matmul(
                            uT[:, T * 128:(T + 1) * 128],
                            est[:, j * L:(j + 1) * L],
                            cb["ident16"][:, :],
                            start=(i == 0), stop=(i == TOPK - 1))

            def st8(s, d):
                # UF | VF | VFsw fp16 DFTs (the swap is free in the weights)
                uT, vt = d["uT"], d["vt"]
                uTsb = wk.tile([L, HE], F16, tag="uTsb", name="uTsb")
                nc.scalar.copy(uTsb[:], uT[:])
                UV = d["UV"] = psUV.tile([PW, 3 * HE], FP, tag="UV", name="UV")
                nc.tensor.matmul(UV[:, 0:HE], cb["Dq16"][:], uTsb[:])
                nc.tensor.matmul(UV[:, HE:2 * HE], cb["Dq16"][:], vt[:])
                nc.tensor.matmul(UV[:, 2 * HE:3 * HE], cb["Dq16sw"][:], vt[:])

            def st9(s, d):
                # g1 = UF*VF (all-SBUF fp16, 2x), g2 = UF*VFsw with the
                # VFsw half read straight from PSUM (aligned, single op)
                UV = d["UV"]
                uvf = wk.tile([PW, 2 * HE], F16, tag="uvf", name="uvf")
                nc.scalar.copy(uvf[:], UV[:, 0:2 * HE])
                g = d["g"] = wk.tile([PW, 2 * HE], F16, tag="g", name="g")
                nc.vector.tensor_mul(g[:, 0:HE], uvf[:, 0:HE],
                                     uvf[:, HE:2 * HE])
                nc.vector.tensor_mul(g[:, HE:2 * HE], uvf[:, 0:HE],
                                     UV[:, 2 * HE:3 * HE])

            def st10(s, d):
                # inverse DFT of V path (fp16) + vo DMA'd straight from PSUM
                g = d["g"]
                aggp = psAG.tile([L, HE], FP, tag="aggp", name="aggp")
                nc.tensor.matmul(aggp[:], cb["M1g"][:], g[:, 0:HE],
                                 start=True, stop=False)
                nc.tensor.matmul(aggp[:], cb["M2g"][:], g[:, HE:2 * HE],
                                 start=False, stop=True)
                aggsb = op.tile([L, HE], F16, tag="aggsb", name="aggsb")
                nc.scalar.copy(aggsb[:], aggp[:])
                nc.sync.dma_start(out=vo.ap()[s], in_=aggsb[:])
                d.clear()

            stages = [st0, st1, st2, st3, st4, st5, st6, st7, st8, st9, st10]
            nst = len(stages)
            # software pipeline, stage-staggered. Emission order within an
            # iteration = downstream urgency: the corr recurrence (st2..st4,
            # st1) first so its PSUM bufs recycle fast; V-path fills gaps.
            order = [2, 3, 4, 1, 5, 0, 10, 9, 8, 7, 6]
            for t in range(n_slices + nst - 1):
                for k in order:
                    s = t - k
                    if 0 <= s < n_slices:
                        stages[k](s, tl[s])

    nc.compile()
    return nc, cf32, cb16


_PROG_CACHE = {}


def _get_program(n_slices):
    if n_slices not in _PROG_CACHE:
        _PROG_CACHE[n_slices] = _build_program(n_slices)
    return _PROG_CACHE[n_slices]


def _prep(queries, keys, values):
    q = np.ascontiguousarray(queries, dtype=np.float32).reshape(NSLICES, L, HE)
    k = np.ascontiguousarray(keys, dtype=np.float32).reshape(NSLICES, L, HE)
    v = np.ascontiguousarray(values, dtype=np.float32).reshape(NSLICES, L, HE)
    total = S * NCORES
    pad = total - NSLICES
    if pad:
        z = np.zeros((pad, L, HE), np.float32)
        q = np.concatenate([q, z], 0)
        k = np.concatenate([k, z], 0)
        v = np.concatenate([v, z], 0)
    qh = q.astype(F16_NP)
    ql = (q - qh.astype(np.float32)).astype(F16_NP)
    kh = k.astype(F16_NP)
    kl = (k - kh.astype(np.float32)).astype(F16_NP)
    qkhl = np.concatenate([qh, ql, kh, kl], axis=2)  # [total, L, 4*HE]
    return np.ascontiguousarray(qkhl), v.astype(F16_NP)


def _make_in_maps(qkhl, v16, cf32, cb16, n_slices):
    in_maps = []
    for c in range(NCORES):
        m = {"qk": qkhl[c * n_slices:(c + 1) * n_slices],
             "vs": v16[c * n_slices:(c + 1) * n_slices]}
        m.update(cf32)
        m.update(cb16)
        in_maps.append(m)
    return in_maps


def kernel(queries, keys, values, attn_mask=None):
    qkhl, v16 = _prep(queries, keys, values)
    nc, cf32, cb16 = _get_program(S)
    in_maps = _make_in_maps(qkhl, v16, cf32, cb16, S)
    res = run_bass_kernel_spmd(nc, in_maps, core_ids=list(range(NCORES)))
    coT = np.concatenate([r["co"] for r in res.results], 0)[:NSLICES]
    agg = np.concatenate([r["vo"] for r in res.results], 0)[:NSLICES]
    # coT: [n, 128, NT*96] fp16, corrT[p, T*96+t] = corr[t, T*128+p]
    corr = (coT.reshape(NSLICES, 128, NT, L)
            .transpose(0, 3, 2, 1)            # [n, L, NT, 128]
            .reshape(NSLICES, L, HE))
    return (agg.reshape(B, N, L, H, E).astype(np.float32),
            corr.reshape(B, N, L, H, E).astype(np.float32))
